# revision 63
# baseline (speedup 1.0000x reference)
"""DeepSeek-style MoE (32 routed experts, top-8, grouped routing, 2 shared experts)
on 8 Trainium2 NeuronCores via Bass/Tile.

Strategy (expert-parallel, load-balanced):
- Host computes the routing (sigmoid gate + grouped top-k, bit-matching the
  reference via jax-on-CPU) and gathers each expert's tokens.
- Experts with more than SPLIT_Q tokens are split into virtual pieces; pieces
  are sorted by size, piece of rank r goes to core r%8, slot r//8. All cores
  run the SAME program: n_slots slots with static capacities equal to the
  per-slot max piece size (rounded up to 128), so the kernel is SPMD-static
  while the work stays balanced across cores (sum of caps ~= 4736 vs the 4096
  ideal for the seed-0 routing).
- Per slot the device computes the expert MLP on transposed activations
  entirely with fp32r matmuls (full PE rate at moving dim >= 256, ~1e-3 max
  rel err):
      gu^T = w_gate_up^T @ x^T               (PSUM, 16 K-chunks over H)
      h    = silu(gu_gate) * gu_up           (ACT silu + DVE mul, fp32r SBUF)
      y^T  = w_down^T @ h                    (PSUM -> ACT copy -> DRAM)
  Inputs (xg/wgu) stream on the SP HWDGE queue, phase-B weights and outputs
  on the ACT HWDGE queue, in small per-chunk DMAs, so the serial DMA fabric
  interleaves streams and the next slot's prefetch is never head-of-line
  blocked.
- The shared MLP runs 2-way tensor-parallel (intermediate dim) x 4-way
  data-parallel (tokens).
- Host combines: out = sum of shared partials + scatter-add of
  2.5 * topk_w * y rows per expert piece.

Self-contained: only numpy/jax/concourse imports, shapes hardcoded.
"""
import numpy as np

import concourse.bass as bass
import concourse.mybir as mybir
import concourse.tile as tile
from concourse.bass_utils import run_bass_kernel_spmd

F32 = mybir.dt.float32
F32R = mybir.dt.float32r

T, H, E, I = 4096, 2048, 32, 1024
TOP_K, N_GROUP, TOPK_GROUP = 8, 8, 4
SI = 2048
ROUTED_SCALING = 2.5
N_CORES = 8
SPLIT_Q = 1152   # experts with more tokens are split into virtual experts
MAX_CHUNK = 1152
# shared expert: SHARED_TP-way split of the intermediate dim x SHARED_DP-way
# split of the tokens (SHARED_TP * SHARED_DP == N_CORES)
SHARED_TP = 2
SHARED_DP = 4
S_TOK = T // SHARED_DP      # tokens per core for the shared MLP
S_SI = SI // SHARED_TP      # intermediate slice per core

_HHC = H // 128   # 16 k-chunks over H
_IC = I // 128    # 8 chunks over I
_SIC = SI // 128  # 16 chunks over SI


# ---------------------------------------------------------------- host routing
def _grouped_topk_host(hidden_states, gate_w, gate_bias):
    """Bit-match the reference's jax fp32 routing, on the CPU backend."""
    import jax

    try:
        jax.config.update("jax_platforms", "axon,cpu")
    except Exception:
        pass
    import jax.numpy as jnp

    cpu = jax.devices("cpu")[0]
    with jax.default_device(cpu):
        hs = jnp.asarray(hidden_states)
        gw = jnp.asarray(gate_w)
        bias = jnp.asarray(gate_bias)
        router_logits = hs @ gw
        scores = jax.nn.sigmoid(router_logits)
        sc = scores + bias[None, :]
        t = sc.shape[0]
        g = sc.reshape(t, N_GROUP, E // N_GROUP)
        group_scores = jax.lax.top_k(g, 2)[0].sum(-1)
        grp_idx = jax.lax.top_k(group_scores, TOPK_GROUP)[1]
        grp_mask = jnp.zeros((t, N_GROUP), sc.dtype).at[
            jnp.arange(t)[:, None], grp_idx].set(1.0)
        tok_mask = jnp.repeat(grp_mask, E // N_GROUP, axis=1)
        masked = jnp.where(tok_mask > 0, sc, -jnp.inf)
        topk_ids = jax.lax.top_k(masked, TOP_K)[1]
        w = jnp.take_along_axis(scores, topk_ids, axis=1)
        w = w / (w.sum(-1, keepdims=True) + 1e-20)
        return np.asarray(w), np.asarray(topk_ids)


def _roundup(x, m):
    return -(-x // m) * m


def _chunk_sizes(cap):
    """Split cap (multiple of 128, >=256) into chunks <= MAX_CHUNK, each a
    multiple of 128 and >= 256."""
    out = []
    rem = cap
    while rem > MAX_CHUNK:
        take = MAX_CHUNK if rem - MAX_CHUNK >= 256 else MAX_CHUNK - 256
        out.append(take)
        rem -= take
    out.append(rem)
    return out


def _n_tiles(chunk):
    """Split chunk (any int >= 256) into matmul N-tiles in [256, 512]
    (fp32r full rate needs a moving dim >= 256)."""
    out = []
    rem = chunk
    while rem > 512:
        take = 512 if rem - 512 >= 256 else rem - 256
        take -= take % 8
        out.append(take)
        rem -= take
    assert 256 <= rem <= 512, rem
    out.append(rem)
    return out


# ---------------------------------------------------------------- bass program
def _build_nc(caps, include_routed=True, include_shared=True):
    nc = bass.Bass()
    CT = sum(caps)
    n_slots = len(caps)

    xg_d = nc.dram_tensor("xg", [H, CT], F32R, kind="ExternalInput")
    wgu_d = nc.dram_tensor("wgu", [n_slots, I // 128, H, 256], F32R, kind="ExternalInput")
    wdn_d = nc.dram_tensor("wdn", [n_slots, H // 128, I, 128], F32R, kind="ExternalInput")
    sgu_d = nc.dram_tensor("sgu", [S_SI // 128, H, 256], F32R, kind="ExternalInput")
    sdn_d = nc.dram_tensor("sdn", [H // 128, S_SI, 128], F32R, kind="ExternalInput")
    xts_d = nc.dram_tensor("xts", [H, S_TOK], F32R, kind="ExternalInput")
    y_d = nc.dram_tensor("y", [H, CT], F32, kind="ExternalOutput")
    ys_d = nc.dram_tensor("ys", [H, S_TOK], F32, kind="ExternalOutput")

    xg_v = xg_d.rearrange("(k p) n -> p k n", p=128)
    xts_v = xts_d.rearrange("(k p) n -> p k n", p=128)
    y_v = y_d.rearrange("(m p) n -> m p n", p=128)
    ys_v = ys_d.rearrange("(m p) n -> m p n", p=128)

    silu = mybir.ActivationFunctionType.Silu
    copy_fn = mybir.ActivationFunctionType.Copy

    from contextlib import ExitStack

    with tile.TileContext(nc) as tc, ExitStack() as ctx:
        xg_pool = ctx.enter_context(tc.tile_pool(name="xgp", bufs=1))
        w_pool = ctx.enter_context(tc.tile_pool(name="wp", bufs=2))
        dn_pool = ctx.enter_context(tc.tile_pool(name="dnp", bufs=3))
        h_pool = ctx.enter_context(tc.tile_pool(name="hp", bufs=2))
        y_pool = ctx.enter_context(tc.tile_pool(name="yp", bufs=2))
        psA = ctx.enter_context(tc.tile_pool(name="psA", bufs=3, space="PSUM"))
        psB = ctx.enter_context(tc.tile_pool(name="psB", bufs=2, space="PSUM"))

        first_chunk = [True]

        def mlp_chunk(gu_pair_srcs, dn_m_srcs, x_src, y_dst, cw, n_pairs, n_kA):
            """One token-chunk of one expert MLP.

            gu_pair_srcs[mp]: DRAM AP [p, n_kA, 256] (gate|up cols of pair mp)
            dn_m_srcs[m]:     DRAM AP [p, n_pairs, 128]
            x_src:            DRAM AP [p, n_kA, cw]
            y_dst[m]:         DRAM AP [p, cw]
            """
            tiles = _n_tiles(cw)

            def wp_load(mp):
                wp = w_pool.tile([128, n_kA, 256], F32R, tag="w", name="wpt")
                qk = n_kA // 4
                for q in range(4):
                    nc.sync.dma_start(out=wp[:, q * qk:(q + 1) * qk],
                                      in_=gu_pair_srcs[mp][:, q * qk:(q + 1) * qk])
                return wp

            # per-k-chunk xg DMAs: keeps single transfers small so the serial
            # DMA fabric interleaves them with the previous slot's dn/y stream.
            # For the program's first chunk, issue only the critical quarter of
            # wgu pair 0 and xg chunk 0 first so the first matmul starts ASAP.
            if first_chunk[0]:
                first_chunk[0] = False
                wp_next = w_pool.tile([128, n_kA, 256], F32R, tag="w", name="wpt")
                qk = n_kA // 4
                nc.sync.dma_start(out=wp_next[:, :qk], in_=gu_pair_srcs[0][:, :qk])
                xt = xg_pool.tile([128, n_kA, cw], F32R, tag="xg", name="xt")
                nc.sync.dma_start(out=xt[:, 0], in_=x_src[:, 0])
                for q in range(1, 4):
                    nc.sync.dma_start(out=wp_next[:, q * qk:(q + 1) * qk],
                                      in_=gu_pair_srcs[0][:, q * qk:(q + 1) * qk])
                for k in range(1, n_kA):
                    nc.sync.dma_start(out=xt[:, k], in_=x_src[:, k])
            else:
                wp_next = wp_load(0)
                xt = xg_pool.tile([128, n_kA, cw], F32R, tag="xg", name="xt")
                for k in range(n_kA):
                    nc.sync.dma_start(out=xt[:, k], in_=x_src[:, k])
            ht = h_pool.tile([128, n_pairs, cw], F32R, tag="h", name="ht")
            for mp in range(n_pairs):
                wp = wp_next
                if mp + 1 < n_pairs:
                    wp_next = wp_load(mp + 1)
                off = 0
                for nt in tiles:
                    g = psA.tile([128, 512], F32, tag="g", name="gps", bufs=4)[:, :nt]
                    u = psA.tile([128, 512], F32, tag="u", name="ups", bufs=2)[:, :nt]
                    for k in range(n_kA):
                        nc.tensor.matmul(
                            g, wp[:, k, 0:128], xt[:, k, off:off + nt],
                            start=(k == 0), stop=(k == n_kA - 1))
                        nc.tensor.matmul(
                            u, wp[:, k, 128:256], xt[:, k, off:off + nt],
                            start=(k == 0), stop=(k == n_kA - 1))
                    hslice = ht[:, mp, off:off + nt]
                    nc.scalar.activation(hslice, g, silu)
                    nc.vector.tensor_mul(hslice, hslice, u)
                    off += nt
            # phase B weights and outputs ride the ACT HWDGE queue so the SP
            # queue stays free for the next slot's xg/wgu prefetch; dn loads
            # are emitted one m'-tile ahead so y stores never block them
            def dn_load(m):
                dt_ = dn_pool.tile([128, n_pairs, 128], F32R, tag="dn", name="dnt")
                nc.scalar.dma_start(out=dt_[:], in_=dn_m_srcs[m])
                return dt_
            dn_q = [dn_load(0), dn_load(1)]
            for m in range(_HHC):
                dt_ = dn_q.pop(0)
                if m + 2 < _HHC:
                    dn_q.append(dn_load(m + 2))
                yt = y_pool.tile([128, cw], F32, tag="y", name="yt")
                off = 0
                for nt in tiles:
                    py = psB.tile([128, 512], F32, tag="py", name="pyps")[:, :nt]
                    for k in range(n_pairs):
                        nc.tensor.matmul(
                            py, dt_[:, k, :], ht[:, k, off:off + nt],
                            start=(k == 0), stop=(k == n_pairs - 1))
                    nc.scalar.activation(yt[:, off:off + nt], py, copy_fn)
                    off += nt
                nc.scalar.dma_start(out=y_dst[m], in_=yt[:])

        # shared expert: TP slice of the intermediate dim x DP slice of tokens
        sgu_srcs = [sgu_d[mp].rearrange("(k p) c -> p k c", p=128)
                    for mp in range(S_SI // 128)]
        sdn_srcs = [sdn_d[m].rearrange("(k p) c -> p k c", p=128)
                    for m in range(_HHC)]

        def shared_chunk(t0, t1):
            mlp_chunk(
                sgu_srcs,
                sdn_srcs,
                xts_v[:, :, t0:t1],
                [ys_v[m][:, t0:t1] for m in range(_HHC)],
                t1 - t0, n_pairs=S_SI // 128, n_kA=_HHC,
            )

        # routed slots, with a shared-expert chunk interleaved mid-way to give
        # the scheduler independent PE work across slot junctions
        off = 0
        for s in range(n_slots if include_routed else 0):
            cap = caps[s]
            gu_srcs = [wgu_d[s, mp].rearrange("(k p) c -> p k c", p=128)
                       for mp in range(_IC)]
            for cw in _chunk_sizes(cap):
                dn_srcs = [wdn_d[s, m].rearrange("(k p) c -> p k c", p=128)
                           for m in range(_HHC)]
                o = off
                mlp_chunk(
                    gu_srcs,
                    dn_srcs,
                    xg_v[:, :, o:o + cw],
                    [y_v[m][:, o:o + cw] for m in range(_HHC)],
                    cw, n_pairs=_IC, n_kA=_HHC,
                )
                off += cw
        if include_shared:
            shared_chunk(0, S_TOK)

    _split_wide_waits(nc)
    return nc


# ------------------------------------------------------- walrus wait-limit fix
def _split_wide_waits(nc):
    """walrus codegen allows only 1 sync wait on fused 4-byte matmuls (and few
    on ctrl ops). Hoist extra waits into single-wait NoOps on the same engine."""
    n = 0
    for f in nc.m.functions:
        for bb in f.blocks:
            il = bb.instructions
            i = 0
            while i < len(il):
                inst = il[i]
                si = inst.sync_info
                waits = list(si.on_wait) if si and si.on_wait else []
                cap = 1
                if len(waits) > cap:
                    inst.sync_info = mybir.SyncInfo(
                        on_wait=waits[:cap], on_update=list(si.on_update or []))
                    nops = [
                        mybir.InstNoOp(
                            name=nc.get_next_instruction_name(),
                            sync_info=mybir.SyncInfo(on_wait=[w], on_update=[]),
                            bass_nofuse=True,
                            engine=inst.engine,
                        )
                        for w in waits[cap:]
                    ]
                    il[i:i] = nops
                    i += len(nops)
                    n += len(nops)
                i += 1
    return n


# ------------------------------------------------------------------- assembly
def plan(topk_w, topk_ids):
    """Work assignment: split big experts into virtual pieces (<= SPLIT_Q
    tokens), sort pieces by size, piece of rank r -> core r % 8, slot r // 8.
    Slot capacities are the per-slot maxima; slots are ordered smallest-first.
    Returns (caps, assign, tok_of) where assign[s][c] = (expert, start, n)."""
    counts = np.bincount(topk_ids.ravel(), minlength=E)
    tok_of = [np.nonzero(topk_ids == e) for e in range(E)]
    live = [e for e in range(E) if counts[e] > 0]
    cs = [int(counts[e]) for e in live]

    def split_sizes(c, k):
        return [c // k + (1 if i < c % k else 0) for i in range(k)]

    def sum_caps(ks):
        sizes = sorted((s for c, k in zip(cs, ks) for s in split_sizes(c, k)),
                       reverse=True)
        ns = -(-len(sizes) // N_CORES)
        return sum(max(256, _roundup(sizes[N_CORES * s], 8)) for s in range(ns))

    # start: balanced split of everything above SPLIT_Q, then greedily move
    # splits between experts while it lowers the total static capacity
    ks = [-(-c // SPLIT_Q) for c in cs]
    budget = _roundup(sum(ks), N_CORES) // 1  # piece budget: fill current slots
    best = sum_caps(ks)
    improved = True
    while improved:
        improved = False
        for a in range(len(cs)):
            if sum(ks) < budget:
                ks[a] += 1
                v = sum_caps(ks)
                if v < best:
                    best, improved = v, True
                    continue
                ks[a] -= 1
            for b in range(len(cs)):
                if b == a or ks[b] < 2:
                    continue
                if -(-cs[b] // (ks[b] - 1)) > SPLIT_Q:
                    continue  # keep every piece (and so every cap) <= one chunk
                ks[a] += 1
                ks[b] -= 1
                v = sum_caps(ks)
                if v < best:
                    best, improved = v, True
                    break
                ks[a] -= 1
                ks[b] += 1

    pieces = []
    for e, c, k in zip(live, cs, ks):
        st = 0
        for n in split_sizes(c, k):
            pieces.append((e, st, n))
            st += n
    pieces.sort(key=lambda p: -p[2])
    n_slots = -(-len(pieces) // N_CORES)
    pieces += [(0, 0, 0)] * (n_slots * N_CORES - len(pieces))
    slots = [pieces[N_CORES * s:N_CORES * (s + 1)] for s in range(n_slots)]
    slots.sort(key=lambda sl: sl[0][2])  # ascending cap
    # near-exact per-slot maxima; multiple of 8 for matmul/DMA alignment
    caps = [max(256, _roundup(sl[0][2], 8)) for sl in slots]
    return caps, slots, tok_of


def kernel(hidden_states, gate_w, gate_bias, w_gate_up, w_down,
           shared_gate_up, shared_down):
    hs = np.ascontiguousarray(hidden_states, dtype=np.float32)
    topk_w, topk_ids = _grouped_topk_host(hs, gate_w, gate_bias)
    caps, slots, tok_of = plan(topk_w, topk_ids)
    n_slots = len(caps)
    CT = sum(caps)
    offs = np.concatenate([[0], np.cumsum(caps)])[:n_slots]

    w_gate_up = np.asarray(w_gate_up, dtype=np.float32)
    w_down = np.asarray(w_down, dtype=np.float32)
    shared_gate_up = np.asarray(shared_gate_up, dtype=np.float32)
    shared_down = np.asarray(shared_down, dtype=np.float32)

    # shared tensors: per TP-slice of the intermediate dim
    SGU_tp, SDN_tp = [], []
    for tp in range(SHARED_TP):
        base = tp * S_SI
        sgu = np.empty((S_SI // 128, H, 256), np.float32)
        for mp in range(S_SI // 128):
            sgu[mp, :, 0:128] = shared_gate_up[:, base + mp * 128: base + (mp + 1) * 128]
            sgu[mp, :, 128:256] = shared_gate_up[
                :, SI + base + mp * 128: SI + base + (mp + 1) * 128]
        SGU_tp.append(sgu)
        sdn = np.empty((H // 128, S_SI, 128), np.float32)
        for m in range(H // 128):
            sdn[m] = shared_down[base:base + S_SI, m * 128:(m + 1) * 128]
        SDN_tp.append(sdn)
    XTS_dp = [
        np.ascontiguousarray(hs[dp * S_TOK:(dp + 1) * S_TOK].T)
        for dp in range(SHARED_DP)
    ]

    in_maps = []
    for c in range(N_CORES):
        XG = np.zeros((H, CT), np.float32)
        WGU = np.zeros((n_slots, I // 128, H, 256), np.float32)
        WDN = np.zeros((n_slots, H // 128, I, 128), np.float32)
        for s in range(n_slots):
            e, st, n = slots[s][c]
            if n == 0:
                continue
            idx = tok_of[e][0][st:st + n]
            XG[:, offs[s]:offs[s] + n] = hs[idx].T
            wg = w_gate_up[e]
            for mp in range(I // 128):
                WGU[s, mp, :, 0:128] = wg[:, mp * 128:(mp + 1) * 128]
                WGU[s, mp, :, 128:256] = wg[:, I + mp * 128: I + (mp + 1) * 128]
            wd = w_down[e]
            for m in range(H // 128):
                WDN[s, m] = wd[:, m * 128:(m + 1) * 128]
        tp, dp = c // SHARED_DP, c % SHARED_DP
        in_maps.append({
            "xg": XG, "wgu": WGU, "wdn": WDN,
            "sgu": SGU_tp[tp], "sdn": SDN_tp[tp], "xts": XTS_dp[dp],
        })

    nc = _build_nc(caps)
    res = run_bass_kernel_spmd(nc, in_maps, list(range(N_CORES)))

    out = np.zeros((T, H), np.float32)
    for c in range(N_CORES):
        dp = c % SHARED_DP
        out[dp * S_TOK:(dp + 1) * S_TOK] += res.results[c]["ys"].T
    for c in range(N_CORES):
        y = res.results[c]["y"]
        for s in range(n_slots):
            e, st, n = slots[s][c]
            if n == 0:
                continue
            idx = tok_of[e][0][st:st + n]
            kpos = tok_of[e][1][st:st + n]
            wts = topk_w[idx, kpos].astype(np.float32) * ROUTED_SCALING
            out[idx] += wts[:, None] * y[:, offs[s]:offs[s] + n].T
    return out
